# revision 1
# baseline (speedup 1.0000x reference)
"""Trainium2 Bass kernel for single-head dense attention.

Reference computation (all fp32):
    q = x @ Wq.T + bq ; k = x @ Wk.T + bk ; v = x @ Wv.T + bv      # [N, D]
    att = softmax((q @ k.T) / sqrt(128), axis=-1)                  # [N, N]
    out = (att @ v) @ Wo.T + bo + x                                # [N, D]

N = 8192, D = 1024, 8 NeuronCores.  Queries are sharded 8 ways; no
collectives needed.

Algebraic restructure (exact up to fp reassociation):
  * z = q @ k.T = (x Wq^T + bq) Wk x^T + (q . bk) 1^T.  The bk term adds a
    per-row constant, which softmax cancels exactly, so K IS NEVER
    COMPUTED.  Host folds W_qk = Wq^T Wk and b_qk = bq @ Wk; the device
    computes Q'^T = W_qk^T.T @ X_loc^T + b_qk, then S^T = X Q'^T with
    supers of X^T streamed from HBM.
  * att @ (x Wv^T + bv) Wo^T + bo = (att @ x) @ (Wo Wv)^T + (bo + Wo bv):
    the PV matmul consumes x directly (V never computed); host folds
    W_vo = Wo @ Wv and bo_eff = bo + Wo @ bv (exact: att rows sum to 1).

Per-core program (Tile framework):
  phase 1: Q'^T [D, 1024] in float32r (one 1024^3 GEMM on local tokens)
  phase 2: flash attention over key supers of 1024 in S^T layout (keys on
           partitions).  Per (super, 512-query block): stage A computes
           S^T chunks [128k, 512q] in float32r and exps them (scale
           folded) into bf16 P^T tiles; stage B runs (att @ x) in bf16
           with P^T chunks as stationary operands, plus a ones-vector
           matmul sharing lhsT for the softmax denominators.
  phase 3: PE-transpose O, @ W_vo^T (float32r), then one fused DVE op
           per tile: out = psum * (1/denom) + x  (row-normalization
           commutes with the output projection).
"""

import sys

if "/opt/trn_rl_repo" not in sys.path:
    sys.path.insert(0, "/opt/trn_rl_repo")

import numpy as np

import concourse.bass as bass
import concourse.tile as tile
from concourse import bacc, mybir
from concourse.masks import make_identity

N = 8192
D = 1024
NCORES = 8
TLOC = N // NCORES  # 1024 tokens per core
SCALE = float(np.sqrt(128.0))
F32 = mybir.dt.float32
F32R = mybir.dt.float32r
BF16 = mybir.dt.bfloat16
FP8 = mybir.dt.float8e4
DR = mybir.MatmulPerfMode.DoubleRow
ActF = mybir.ActivationFunctionType
AluOp = mybir.AluOpType

KSUP = 1024           # keys per attention super-block
NSUP = N // KSUP      # 16
TSUP = 512            # token block in phase 1
QBLK = 512            # query columns per S^T matmul
DC = D // 128         # 8 feature chunks

_PROGRAM_CACHE = {}


def build_program():
    nc = bacc.Bacc("TRN2", target_bir_lowering=False, debug=False,
                   num_devices=NCORES)

    xt_full = nc.dram_tensor("xt_full", [D, N], BF16, kind="ExternalInput")
    x_f8 = nc.dram_tensor("x_f8", [N, D], FP8, kind="ExternalInput")
    xt_loc = nc.dram_tensor("xt_loc", [D, TLOC], F32R, kind="ExternalInput")
    x_loc = nc.dram_tensor("x_loc", [TLOC, D], F32, kind="ExternalInput")
    w_qk = nc.dram_tensor("w_qk", [D, D], F32R, kind="ExternalInput")
    w_vo_t = nc.dram_tensor("w_vo_t", [D, D], BF16, kind="ExternalInput")
    bqk2 = nc.dram_tensor("bqk2", [D, 1], F32, kind="ExternalInput")
    out_ext = nc.dram_tensor("out", [TLOC, D], F32, kind="ExternalOutput")

    with tile.TileContext(nc) as tc:
        import contextlib

        with contextlib.ExitStack() as ctx:
            const = ctx.enter_context(tc.tile_pool(name="const", bufs=1))
            persist = ctx.enter_context(tc.tile_pool(name="persist", bufs=1))

            identity = const.tile([128, 128], BF16)
            make_identity(nc, identity[:])
            zbias = const.tile([128, 1], F32)
            nc.vector.memset(zbias[:], 0.0)
            ones_k8 = const.tile([128, 2, 1], FP8)
            nc.vector.memset(ones_k8[:], 1.0)
            mbias = const.tile([128, 1], F32)
            nc.vector.memset(mbias[:], -2.0)
            bqk_sb = const.tile([128, DC, 1], F32)
            nc.sync.dma_start(
                bqk_sb[:], bqk2.ap().rearrange("(c p) o -> p c o", p=128))

            # persistent SBUF tensors
            qpt_sb = persist.tile([128, DC, TLOC], BF16)   # Q'^T {ec x q}
            o_sb = persist.tile([128, TLOC // 128, D], BF16)  # att@x {qc x e}
            den_sb = persist.tile([128, TLOC // 128], F32)
            rden_sb = persist.tile([128, TLOC // 128], F32)
            nc.vector.memset(o_sb[:], 0.0)
            nc.vector.memset(den_sb[:], 0.0)

            # attention pools opened before phase 1 so super-0 K/V DMAs
            # get disjoint SBUF addresses and prefetch during the Q' GEMM
            kvp = ctx.enter_context(tc.tile_pool(name="kv", bufs=2))
            ptp = ctx.enter_context(tc.tile_pool(name="pt", bufs=10))

            # ---------------- phase 1: Q'^T (local tokens) ----------------
            with nc.named_scope("p1_qproj"), \
                 tc.tile_pool(name="wqk", bufs=1) as wqkp, \
                 tc.tile_pool(name="xtl", bufs=2) as xtlp, \
                 tc.tile_pool(name="ps1", bufs=4, space="PSUM") as ps1:
                wqk_sb = wqkp.tile([128, DC, D], F32R)  # {ec x e2}
                nc.sync.dma_start(
                    wqk_sb[:], w_qk.ap().rearrange("(c p) d -> p c d", p=128))
                for ts in range(TLOC // TSUP):
                    xt = xtlp.tile([128, DC, TSUP], F32R, tag="xtl")
                    nc.sync.dma_start(
                        xt[:],
                        xt_loc[:, ts * TSUP:(ts + 1) * TSUP].rearrange(
                            "(c p) t -> p c t", p=128))
                    for dc in range(DC):
                        qp = ps1.tile([128, TSUP], F32, tag="qp")
                        for ec in range(DC):
                            nc.tensor.matmul(
                                qp[:],
                                lhsT=wqk_sb[:, ec, dc * 128:dc * 128 + 128],
                                rhs=xt[:, ec, :],
                                start=(ec == 0), stop=(ec == DC - 1))
                        nc.vector.tensor_scalar_add(
                            qpt_sb[:, dc, ts * TSUP:(ts + 1) * TSUP],
                            qp[:], bqk_sb[:, dc, :])

            # ---------------- phase 2: flash attention --------------------
            with nc.named_scope("p2_attn"), \
                 tc.tile_pool(name="pso", bufs=4, space="PSUM") as pso, \
                 tc.tile_pool(name="psst", bufs=2, space="PSUM") as psst, \
                 tc.tile_pool(name="psden", bufs=2, space="PSUM") as psden:
                KC = KSUP // 128  # 4 k-chunks per super
                for s in range(NSUP):
                    k_sb = kvp.tile([128, DC, KSUP], BF16, tag="k")
                    nc.sync.dma_start(
                        k_sb[:],
                        xt_full[:, s * KSUP:(s + 1) * KSUP].rearrange(
                            "(c p) t -> p c t", p=128))
                    v_sb = kvp.tile([128, KSUP // 256, 2, D], FP8, tag="v")
                    nc.sync.dma_start(
                        v_sb[:],
                        x_f8[s * KSUP:(s + 1) * KSUP, :].rearrange(
                            "(g ko p) d -> p g ko d", p=128, ko=2))
                    for qb in range(TLOC // QBLK):
                        # stage A: S^T chunks -> exp(z/s - 2) -> fp8 P^T
                        # planes [Ki, Ko] for DoubleRow (shift cancels in
                        # softmax; keeps exp under e4m3 max 448)
                        pts = []
                        for kc in range(KC):
                            if kc % 2 == 0:
                                pt_t = ptp.tile([128, 2, QBLK], FP8,
                                                tag="pt")
                                pts.append(pt_t)
                            st = psst.tile([128, QBLK], F32, tag="st")
                            for dc in range(DC):
                                nc.tensor.matmul(
                                    st[:],
                                    lhsT=k_sb[:, dc, kc * 128:kc * 128 + 128],
                                    rhs=qpt_sb[:, dc,
                                               qb * QBLK:(qb + 1) * QBLK],
                                    start=(dc == 0), stop=(dc == DC - 1))
                            nc.scalar.activation(
                                pts[kc // 2][:, kc % 2, :], st[:], ActF.Exp,
                                bias=mbias[:, 0:1], scale=1.0 / SCALE)
                        # stage B: (att @ x) + denominators, bf16
                        NG = KSUP // 256  # 256-key DoubleRow groups
                        for half in range(2):
                            for sub in range(QBLK // 128):
                                o_ps = pso.tile([128, 512], F32, tag="ops")
                                if half == 0:
                                    d_ps = psden.tile([128, 1], F32,
                                                      tag="dps")
                                for g in range(NG):
                                    lhs = pts[g][:, :, sub * 128:
                                                 (sub + 1) * 128]
                                    nc.tensor.matmul(
                                        o_ps[:],
                                        lhsT=lhs,
                                        rhs=v_sb[:, g, :, half * 512:
                                                 half * 512 + 512],
                                        start=(g == 0), stop=(g == NG - 1),
                                        perf_mode=DR)
                                    if half == 0:
                                        nc.tensor.matmul(
                                            d_ps[:, 0:1],
                                            lhsT=lhs,
                                            rhs=ones_k8[:, :, 0:1],
                                            start=(g == 0),
                                            stop=(g == NG - 1),
                                            perf_mode=DR)
                                qc = qb * (QBLK // 128) + sub
                                nc.vector.tensor_add(
                                    o_sb[:, qc, half * 512:half * 512 + 512],
                                    o_ps[:],
                                    o_sb[:, qc, half * 512:half * 512 + 512])
                                if half == 0:
                                    nc.vector.tensor_add(
                                        den_sb[:, qc:qc + 1],
                                        d_ps[:, 0:1],
                                        den_sb[:, qc:qc + 1])

            # ---------------- phase 3: out-proj + normalize + residual ----
            with nc.named_scope("p3_out"), \
                 tc.tile_pool(name="wo", bufs=1) as wop, \
                 tc.tile_pool(name="ot", bufs=1) as otp, \
                 tc.tile_pool(name="xr", bufs=2) as xrp, \
                 tc.tile_pool(name="fo", bufs=4) as fop, \
                 tc.tile_pool(name="pst", bufs=4, space="PSUM") as pstp, \
                 tc.tile_pool(name="psf", bufs=4, space="PSUM") as psfp:
                QC = TLOC // 128  # 8
                wo_sb = wop.tile([128, DC, D], BF16)  # {ec x d2}
                nc.sync.dma_start(
                    wo_sb[:],
                    w_vo_t.ap().rearrange("(c p) d -> p c d", p=128))
                ot_sb = otp.tile([128, DC, TLOC], BF16)  # (att@x)^T {ec x q}

                for qc in range(QC):
                    nc.vector.reciprocal(rden_sb[:, qc:qc + 1],
                                         den_sb[:, qc:qc + 1])
                    for dc in range(DC):
                        tp = pstp.tile([128, 128], BF16, tag="tp")
                        nc.tensor.transpose(
                            tp[:], o_sb[:, qc, dc * 128:dc * 128 + 128],
                            identity[:])
                        nc.vector.tensor_copy(
                            ot_sb[:, dc, qc * 128:(qc + 1) * 128], tp[:])
                    xr = xrp.tile([128, D], F32, tag="xr")
                    nc.sync.dma_start(
                        xr[:], x_loc[qc * 128:(qc + 1) * 128, :])
                    for half in range(2):
                        fp = psfp.tile([128, 512], F32, tag="fp")
                        for dc in range(DC):
                            nc.tensor.matmul(
                                fp[:],
                                lhsT=ot_sb[:, dc, qc * 128:(qc + 1) * 128],
                                rhs=wo_sb[:, dc, half * 512:half * 512 + 512],
                                start=(dc == 0), stop=(dc == DC - 1))
                        fo = fop.tile([128, 512], F32, tag="fo")
                        # out = psum * (1/denom) + (x + bo_eff), fused
                        nc.vector.scalar_tensor_tensor(
                            fo[:], fp[:], rden_sb[:, qc:qc + 1],
                            xr[:, half * 512:half * 512 + 512],
                            op0=AluOp.mult, op1=AluOp.add)
                        nc.sync.dma_start(
                            out_ext[qc * 128:(qc + 1) * 128,
                                    half * 512:half * 512 + 512], fo[:])

    nc.compile()
    return nc


def _get_program():
    if "nc" not in _PROGRAM_CACHE:
        _PROGRAM_CACHE["nc"] = build_program()
    return _PROGRAM_CACHE["nc"]


def make_in_maps(x, Wq, bq, Wk, bk, Wv, bv, Wo, bo):
    """Host-side sharding/layout prep and weight folding (constant folding
    of D x D weight products -- all N-sized tensor math runs on device).
    Returns per-core input maps."""
    import ml_dtypes

    x = np.ascontiguousarray(x, dtype=np.float32)
    xt = np.ascontiguousarray(x.T)
    x_f8 = x.astype(ml_dtypes.float8_e4m3fn)
    Wq64 = np.asarray(Wq, np.float64)
    Wk64 = np.asarray(Wk, np.float64)
    Wv64 = np.asarray(Wv, np.float64)
    Wo64 = np.asarray(Wo, np.float64)
    # z = q k^T = (x Wq^T + bq) Wk x^T + (q.bk) 1^T; the bk term is a
    # per-row constant -- softmax cancels it exactly, so K is dropped.
    w_qk = np.ascontiguousarray((Wq64.T @ Wk64).astype(np.float32))
    bqk = (np.asarray(bq, np.float64) @ Wk64).astype(np.float32)
    # att(x Wv^T + bv) Wo^T + bo = (att x)(Wo Wv)^T + (bo + Wo bv),
    # exact because att rows sum to 1 in the on-device normalization.
    w_vo_t = np.ascontiguousarray((Wo64 @ Wv64).T.astype(np.float32))
    boeff = (np.asarray(bo, np.float64)
             + Wo64 @ np.asarray(bv, np.float64)).astype(np.float32)
    in_maps = []
    for c in range(NCORES):
        sl = slice(c * TLOC, (c + 1) * TLOC)
        in_maps.append({
            "xt_full": xt.astype(ml_dtypes.bfloat16),
            "x_f8": x_f8,
            "xt_loc": np.ascontiguousarray(xt[:, sl]),
            "x_loc": np.ascontiguousarray(x[sl, :] + boeff[None, :]),
            "w_qk": w_qk,
            "w_vo_t": w_vo_t.astype(ml_dtypes.bfloat16),
            "bqk2": bqk.reshape(D, 1),
        })
    return in_maps


def kernel(x, Wq, bq, Wk, bk, Wv, bv, Wo, bo, _trace=False):
    from concourse.bass_utils import run_bass_kernel_spmd

    nc = _get_program()
    in_maps = make_in_maps(x, Wq, bq, Wk, bk, Wv, bv, Wo, bo)
    res = run_bass_kernel_spmd(nc, in_maps, list(range(NCORES)),
                               trace=_trace)
    out = np.concatenate([res.results[c]["out"] for c in range(NCORES)],
                         axis=0)
    if _trace:
        kernel.last_results = res
    return out



# revision 9
# speedup vs baseline: 1.5586x; 1.5586x over previous
"""Trainium2 Bass kernel for single-head dense attention.

Reference computation (all fp32):
    q = x @ Wq.T + bq ; k = x @ Wk.T + bk ; v = x @ Wv.T + bv      # [N, D]
    att = softmax((q @ k.T) / sqrt(128), axis=-1)                  # [N, N]
    out = (att @ v) @ Wo.T + bo + x                                # [N, D]

N = 8192, D = 1024, 8 NeuronCores.  Queries are sharded 8 ways; no
collectives needed.

Algebraic restructure (exact up to fp reassociation):
  * z = q @ k.T = (x Wq^T + bq) Wk x^T + (q . bk) 1^T.  The bk term adds a
    per-row constant, which softmax cancels exactly, so K IS NEVER
    COMPUTED.  Host folds W_qk = 32 Wq^T Wk and b_qk = 32 bq @ Wk (the x32
    keeps fp8 weights out of the denormal range; it cancels in softmax via
    the exp input scale).
  * att @ (x Wv^T + bv) Wo^T + bo = (att @ x) @ (Wo Wv)^T + (bo + Wo bv):
    the PV matmul consumes x directly (V never computed); host folds
    W_vo = 32 Wo @ Wv (x32 rescaled away in the output normalization) and
    bo_eff = bo + Wo @ bv (exact: att rows sum to 1).

All big GEMMs run in fp8e4m3 with DoubleRow perf mode (256-deep
contraction per instruction, 2x bf16 MAC throughput on HW).  Softmax
denominators come for free: the fp8 V matrix gets an extra column of
32.0, so column 1024 of the PV output is 32*sum_k(P) and the final
normalization uses 1/(32 den) directly.

Per-core program (Tile framework):
  phase 1: Q'^T [D, 1024] fp8 (DR GEMM on local tokens, psum f32 + bias,
           written as fp8 in (pair, plane) layout for stage A)
  phase 2: flash attention over key supers of 1024 in S^T layout (keys on
           partitions).  Per (super, 512-query block): stage A computes
           S^T chunks [128k, 512q] via fp8 DR and exps them (scale folded,
           -2 shift keeps exp under e4m3 max 448) into fp8 P^T tiles;
           stage B runs (att @ [x | 32]) via fp8 DR with P^T chunks as
           stationary operands; the PSUM is accumulated into bf16 o_sb
           (denominator column included) by the DVE.
  phase 3: PE-transpose O, @ W_vo^T (fp8 DR), then one fused DVE op
           per tile: out = psum * (1/(32 den)) + (x + bo_eff).
"""

import sys

if "/opt/trn_rl_repo" not in sys.path:
    sys.path.insert(0, "/opt/trn_rl_repo")

import numpy as np

import concourse.bass as bass
import concourse.tile as tile
from concourse import bacc, mybir
from concourse.masks import make_identity

N = 8192
D = 1024
NCORES = 8
TLOC = N // NCORES  # 1024 tokens per core
SCALE = float(np.sqrt(128.0))
WSC = 32.0            # fp8 weight pre-scale (denormal avoidance)
F32 = mybir.dt.float32
BF16 = mybir.dt.bfloat16
FP8 = mybir.dt.float8e4
DR = mybir.MatmulPerfMode.DoubleRow
ActF = mybir.ActivationFunctionType
AluOp = mybir.AluOpType

KSUP = 1024           # keys per attention super-block
NSUP = N // KSUP      # 8
TSUP = 512            # token block in phase 1
QBLK = 512            # query columns per S^T matmul
DC = D // 128         # 8 feature chunks
NPAIR = DC // 2       # 4 DoubleRow feature-pair chunks
VW = 1032             # padded V width: 1024 features + 32-col + pad
KC = KSUP // 128      # 8 key chunks per super
NG = KSUP // 256      # 4 DoubleRow key groups per super

_PROGRAM_CACHE = {}


def build_program():
    nc = bacc.Bacc("TRN2", target_bir_lowering=False, debug=False,
                   num_devices=NCORES)

    xt_f8 = nc.dram_tensor("xt_f8", [D, N], FP8, kind="ExternalInput")
    xpad_f8 = nc.dram_tensor("xpad_f8", [N, VW], FP8, kind="ExternalInput")
    xtl_f8 = nc.dram_tensor("xtl_f8", [D, TLOC], FP8, kind="ExternalInput")
    x_loc = nc.dram_tensor("x_loc", [TLOC, D], F32, kind="ExternalInput")
    w_qk8 = nc.dram_tensor("w_qk8", [D, D], FP8, kind="ExternalInput")
    w_vo8 = nc.dram_tensor("w_vo8", [D, D], FP8, kind="ExternalInput")
    bqk2 = nc.dram_tensor("bqk2", [D, 1], F32, kind="ExternalInput")
    out_ext = nc.dram_tensor("out", [TLOC, D], F32, kind="ExternalOutput")

    with tile.TileContext(nc) as tc:
        import contextlib

        with contextlib.ExitStack() as ctx:
            const = ctx.enter_context(tc.tile_pool(name="const", bufs=1))
            persist = ctx.enter_context(tc.tile_pool(name="persist", bufs=1))

            identity = const.tile([128, 128], BF16)
            make_identity(nc, identity[:])
            mbias = const.tile([128, 1], F32)
            nc.vector.memset(mbias[:], -2.0)
            bqk_sb = const.tile([128, DC, 1], F32)
            nc.sync.dma_start(
                bqk_sb[:], bqk2.ap().rearrange("(c p) o -> p c o", p=128))

            # persistent SBUF tensors
            # Q'^T fp8 {feat-pair x plane x q} for DR stage A
            qpt_sb = persist.tile([128, NPAIR, 2, TLOC], FP8)
            # att@[x|32] accumulator {qc x (e,den)}
            o_sb = persist.tile([128, TLOC // 128, 1026], BF16)
            rden_sb = persist.tile([128, TLOC // 128], F32)
            nc.vector.memset(o_sb[:], 0.0)
            # W_vo loaded early so its DMA overlaps phase 1/2
            wvo_sb = persist.tile([128, NPAIR, 2, D], FP8)
            nc.sync.dma_start(
                wvo_sb[:],
                w_vo8.ap().rearrange("(c q p) d -> p c q d", p=128, q=2))

            # attention pools opened before phase 1 so super-0 K/V DMAs
            # get disjoint SBUF addresses and prefetch during the Q' GEMM
            kvp = ctx.enter_context(tc.tile_pool(name="kv", bufs=2))
            ptp = ctx.enter_context(tc.tile_pool(name="pt", bufs=10))

            # ---------------- phase 1: Q'^T (local tokens) ----------------
            with nc.named_scope("p1_qproj"), \
                 tc.tile_pool(name="wqk", bufs=1) as wqkp, \
                 tc.tile_pool(name="xtl", bufs=2) as xtlp, \
                 tc.tile_pool(name="ps1", bufs=4, space="PSUM") as ps1:
                wqk_sb = wqkp.tile([128, NPAIR, 2, D], FP8)
                nc.sync.dma_start(
                    wqk_sb[:],
                    w_qk8.ap().rearrange("(c q p) d -> p c q d", p=128, q=2))
                for ts in range(TLOC // TSUP):
                    xt = xtlp.tile([128, NPAIR, 2, TSUP], FP8, tag="xtl")
                    nc.sync.dma_start(
                        xt[:],
                        xtl_f8[:, ts * TSUP:(ts + 1) * TSUP].rearrange(
                            "(c q p) t -> p c q t", p=128, q=2))
                    for dc in range(DC):
                        qp = ps1.tile([128, TSUP], F32, tag="qp")
                        for pr in range(NPAIR):
                            nc.tensor.matmul(
                                qp[:],
                                lhsT=wqk_sb[:, pr, :,
                                            dc * 128:dc * 128 + 128],
                                rhs=xt[:, pr, :, :],
                                start=(pr == 0), stop=(pr == NPAIR - 1),
                                perf_mode=DR)
                        nc.vector.tensor_scalar_add(
                            qpt_sb[:, dc // 2, dc % 2,
                                   ts * TSUP:(ts + 1) * TSUP],
                            qp[:], bqk_sb[:, dc, :])

            # ---------------- phase 2: flash attention --------------------
            # stage-B output chunks: each must fit one PSUM bank (<=512
            # fp32); the last chunk carries the denominator column
            OCH = [(0, 512), (512, 768), (768, 1025)]
            with nc.named_scope("p2_attn"), \
                 tc.tile_pool(name="pso0", bufs=2, space="PSUM") as pso0, \
                 tc.tile_pool(name="pso1", bufs=2, space="PSUM") as pso1, \
                 tc.tile_pool(name="pso2", bufs=2, space="PSUM") as pso2, \
                 tc.tile_pool(name="psst", bufs=2, space="PSUM") as psst:
                opools = [pso0, pso1, pso2]
                for s in range(NSUP):
                    k_sb = kvp.tile([128, NPAIR, 2, KSUP], FP8, tag="k")
                    nc.sync.dma_start(
                        k_sb[:],
                        xt_f8[:, s * KSUP:(s + 1) * KSUP].rearrange(
                            "(c q p) t -> p c q t", p=128, q=2))
                    v_sb = kvp.tile([128, NG, 2, VW], FP8, tag="v")
                    nc.sync.dma_start(
                        v_sb[:],
                        xpad_f8[s * KSUP:(s + 1) * KSUP, :].rearrange(
                            "(g ko p) d -> p g ko d", p=128, ko=2))
                    for qb in range(TLOC // QBLK):
                        # stage A: S^T chunks (fp8 DR) -> exp(z/(32 s) - 2)
                        # -> fp8 P^T planes [Ki, Ko] (shift cancels in
                        # softmax; keeps exp under e4m3 max 448)
                        pts = []
                        for kc in range(KC):
                            if kc % 2 == 0:
                                pt_t = ptp.tile([128, 2, QBLK], FP8,
                                                tag="pt")
                                pts.append(pt_t)
                            st = psst.tile([128, QBLK], F32, tag="st")
                            for pr in range(NPAIR):
                                nc.tensor.matmul(
                                    st[:],
                                    lhsT=k_sb[:, pr, :,
                                              kc * 128:kc * 128 + 128],
                                    rhs=qpt_sb[:, pr, :,
                                               qb * QBLK:(qb + 1) * QBLK],
                                    start=(pr == 0), stop=(pr == NPAIR - 1),
                                    perf_mode=DR)
                            nc.scalar.activation(
                                pts[kc // 2][:, kc % 2, :], st[:], ActF.Exp,
                                bias=mbias[:, 0:1], scale=1.0 / (WSC * SCALE))
                        # stage B: att @ [x | 32]; column 1024 of the PSUM
                        # is 32*sum(P) = the softmax denominator
                        for sub in range(QBLK // 128):
                            qc = qb * (QBLK // 128) + sub
                            for ci, (w0, w1) in enumerate(OCH):
                                o_ps = opools[ci].tile(
                                    [128, w1 - w0], F32,
                                    tag="ops%d" % ci)
                                for g in range(NG):
                                    nc.tensor.matmul(
                                        o_ps[:],
                                        lhsT=pts[g][:, :, sub * 128:
                                                    (sub + 1) * 128],
                                        rhs=v_sb[:, g, :, w0:w1],
                                        start=(g == 0), stop=(g == NG - 1),
                                        perf_mode=DR)
                                nc.vector.tensor_add(
                                    o_sb[:, qc, w0:w1], o_ps[:],
                                    o_sb[:, qc, w0:w1])

            # ---------------- phase 3: out-proj + normalize + residual ----
            with nc.named_scope("p3_out"), \
                 tc.tile_pool(name="ot", bufs=1) as otp, \
                 tc.tile_pool(name="xr", bufs=2) as xrp, \
                 tc.tile_pool(name="fo", bufs=4) as fop, \
                 tc.tile_pool(name="pst", bufs=4, space="PSUM") as pstp, \
                 tc.tile_pool(name="psf", bufs=4, space="PSUM") as psfp:
                QC = TLOC // 128  # 8
                ot_sb = otp.tile([128, NPAIR, 2, TLOC], FP8)  # (att@x)^T

                for qc in range(QC):
                    nc.vector.reciprocal(rden_sb[:, qc:qc + 1],
                                         o_sb[:, qc, 1024:1025])
                    for dc in range(DC):
                        tp = pstp.tile([128, 128], BF16, tag="tp")
                        nc.tensor.transpose(
                            tp[:], o_sb[:, qc, dc * 128:dc * 128 + 128],
                            identity[:])
                        # x1/8 keeps the fp8 copy under the e4m3 max of
                        # 448 (raw O reaches ~450); the den column of
                        # 4.0 (= 32/8) compensates exactly
                        nc.vector.tensor_scalar_mul(
                            ot_sb[:, dc // 2, dc % 2,
                                  qc * 128:(qc + 1) * 128], tp[:], 0.125)
                    xr = xrp.tile([128, D], F32, tag="xr")
                    nc.sync.dma_start(
                        xr[:], x_loc[qc * 128:(qc + 1) * 128, :])
                    for half in range(2):
                        fp = psfp.tile([128, 512], F32, tag="fp")
                        for pr in range(NPAIR):
                            nc.tensor.matmul(
                                fp[:],
                                lhsT=ot_sb[:, pr, :,
                                           qc * 128:(qc + 1) * 128],
                                rhs=wvo_sb[:, pr, :,
                                           half * 512:half * 512 + 512],
                                start=(pr == 0), stop=(pr == NPAIR - 1),
                                perf_mode=DR)
                        fo = fop.tile([128, 512], F32, tag="fo")
                        # psum = (O/8) @ (32 W_vo)^T = 4 O @ W_vo^T and
                        # rden = 1/(4 den), so out = psum*rden + x, fused
                        nc.vector.scalar_tensor_tensor(
                            fo[:], fp[:], rden_sb[:, qc:qc + 1],
                            xr[:, half * 512:half * 512 + 512],
                            op0=AluOp.mult, op1=AluOp.add)
                        nc.sync.dma_start(
                            out_ext[qc * 128:(qc + 1) * 128,
                                    half * 512:half * 512 + 512], fo[:])

    nc.compile()
    return nc


def _get_program():
    if "nc" not in _PROGRAM_CACHE:
        _PROGRAM_CACHE["nc"] = build_program()
    return _PROGRAM_CACHE["nc"]


def make_in_maps(x, Wq, bq, Wk, bk, Wv, bv, Wo, bo):
    """Host-side sharding/layout prep and weight folding (constant folding
    of D x D weight products -- all N-sized tensor math runs on device).
    Returns per-core input maps."""
    import ml_dtypes

    f8 = ml_dtypes.float8_e4m3fn
    x = np.ascontiguousarray(x, dtype=np.float32)
    x_f8 = x.astype(f8)
    xt_f8 = np.ascontiguousarray(x_f8.T)
    xpad_f8 = np.zeros((N, VW), dtype=f8)
    xpad_f8[:, :D] = x_f8
    xpad_f8[:, D] = f8(WSC / 8.0)  # den column; matches the /8 O scaling
    Wq64 = np.asarray(Wq, np.float64)
    Wk64 = np.asarray(Wk, np.float64)
    Wv64 = np.asarray(Wv, np.float64)
    Wo64 = np.asarray(Wo, np.float64)
    # z = q k^T = (x Wq^T + bq) Wk x^T + (q.bk) 1^T; the bk term is a
    # per-row constant -- softmax cancels it exactly, so K is dropped.
    w_qk8 = (WSC * (Wq64.T @ Wk64)).astype(np.float32).astype(f8)
    bqk = (WSC * (np.asarray(bq, np.float64) @ Wk64)).astype(np.float32)
    # att(x Wv^T + bv) Wo^T + bo = (att x)(Wo Wv)^T + (bo + Wo bv),
    # exact because att rows sum to 1 in the on-device normalization.
    w_vo8 = np.ascontiguousarray(
        (WSC * (Wo64 @ Wv64)).T.astype(np.float32)).astype(f8)
    boeff = (np.asarray(bo, np.float64)
             + Wo64 @ np.asarray(bv, np.float64)).astype(np.float32)
    in_maps = []
    for c in range(NCORES):
        sl = slice(c * TLOC, (c + 1) * TLOC)
        in_maps.append({
            "xt_f8": xt_f8,
            "xpad_f8": xpad_f8,
            "xtl_f8": np.ascontiguousarray(xt_f8[:, sl]),
            "x_loc": np.ascontiguousarray(x[sl, :] + boeff[None, :]),
            "w_qk8": w_qk8,
            "w_vo8": w_vo8,
            "bqk2": bqk.reshape(D, 1),
        })
    return in_maps


def kernel(x, Wq, bq, Wk, bk, Wv, bv, Wo, bo, _trace=False):
    from concourse.bass_utils import run_bass_kernel_spmd

    nc = _get_program()
    in_maps = make_in_maps(x, Wq, bq, Wk, bk, Wv, bv, Wo, bo)
    res = run_bass_kernel_spmd(nc, in_maps, list(range(NCORES)),
                               trace=_trace)
    out = np.concatenate([res.results[c]["out"] for c in range(NCORES)],
                         axis=0)
    if _trace:
        kernel.last_results = res
    return out


# revision 16
# speedup vs baseline: 1.5823x; 1.0152x over previous
"""Trainium2 Bass kernel for single-head dense attention.

Reference computation (all fp32):
    q = x @ Wq.T + bq ; k = x @ Wk.T + bk ; v = x @ Wv.T + bv      # [N, D]
    att = softmax((q @ k.T) / sqrt(128), axis=-1)                  # [N, N]
    out = (att @ v) @ Wo.T + bo + x                                # [N, D]

N = 8192, D = 1024, 8 NeuronCores.  Queries are sharded 8 ways; no
collectives needed.

Algebraic restructure (exact up to fp reassociation):
  * z = q @ k.T = (x Wq^T + bq) Wk x^T + (q . bk) 1^T.  The bk term adds a
    per-row constant, which softmax cancels exactly, so K IS NEVER
    COMPUTED.  Host folds W_qk = 32 Wq^T Wk and b_qk = 32 bq @ Wk (the x32
    keeps fp8 weights out of the denormal range; it cancels in softmax via
    the exp input scale).
  * att @ (x Wv^T + bv) Wo^T + bo = (att @ x) @ (Wo Wv)^T + (bo + Wo bv):
    the PV matmul consumes x directly (V never computed); host folds
    W_vo = 32 Wo @ Wv (x32 rescaled away in the output normalization) and
    bo_eff = bo + Wo @ bv (exact: att rows sum to 1).

All big GEMMs run in fp8e4m3 with DoubleRow perf mode (256-deep
contraction per instruction, 2x bf16 MAC throughput on HW).  Softmax
denominators come for free: the fp8 V matrix gets an extra column of
32.0, so column 1024 of the PV output is 32*sum_k(P) and the final
normalization uses 1/(32 den) directly.

Per-core program (Tile framework):
  phase 1: Q'^T [D, 1024] fp8 (DR GEMM on local tokens, psum f32 + bias,
           written as fp8 in (pair, plane) layout for stage A)
  phase 2: flash attention over key supers of 1024 in S^T layout (keys on
           partitions).  Per (super, 512-query block): stage A computes
           S^T chunks [128k, 512q] via fp8 DR and exps them (scale folded,
           -2 shift keeps exp under e4m3 max 448) into fp8 P^T tiles;
           stage B runs (att @ [x | 32]) via fp8 DR with P^T chunks as
           stationary operands; the PSUM is accumulated into bf16 o_sb
           (denominator column included) by the DVE.
  phase 3: PE-transpose O, @ W_vo^T (fp8 DR), then one fused DVE op
           per tile: out = psum * (1/(32 den)) + (x + bo_eff).
"""

import sys

if "/opt/trn_rl_repo" not in sys.path:
    sys.path.insert(0, "/opt/trn_rl_repo")

import numpy as np

import concourse.bass as bass
import concourse.tile as tile
from concourse import bacc, mybir
from concourse.masks import make_identity

N = 8192
D = 1024
NCORES = 8
TLOC = N // NCORES  # 1024 tokens per core
SCALE = float(np.sqrt(128.0))
WSC = 32.0            # fp8 weight pre-scale (denormal avoidance)
F32 = mybir.dt.float32
BF16 = mybir.dt.bfloat16
FP8 = mybir.dt.float8e4
DR = mybir.MatmulPerfMode.DoubleRow
ActF = mybir.ActivationFunctionType
AluOp = mybir.AluOpType

KSUP = 1024           # keys per attention super-block
NSUP = N // KSUP      # 8
TSUP = 512            # token block in phase 1
QBLK = 512            # query columns per S^T matmul
DC = D // 128         # 8 feature chunks
NPAIR = DC // 2       # 4 DoubleRow feature-pair chunks
VW = 1032             # padded V width: 1024 features + 32-col + pad
KC = KSUP // 128      # 8 key chunks per super
NG = KSUP // 256      # 4 DoubleRow key groups per super

_PROGRAM_CACHE = {}


def build_program():
    nc = bacc.Bacc("TRN2", target_bir_lowering=False, debug=False,
                   num_devices=NCORES)

    xt_f8 = nc.dram_tensor("xt_f8", [D, N], FP8, kind="ExternalInput")
    xpad_f8 = nc.dram_tensor("xpad_f8", [N, VW], FP8, kind="ExternalInput")
    xtl_f8 = nc.dram_tensor("xtl_f8", [D, TLOC], FP8, kind="ExternalInput")
    x_loc = nc.dram_tensor("x_loc", [TLOC, D], F32, kind="ExternalInput")
    w_qk8 = nc.dram_tensor("w_qk8", [D, D], FP8, kind="ExternalInput")
    w_vo8 = nc.dram_tensor("w_vo8", [D, D], FP8, kind="ExternalInput")
    bqk2 = nc.dram_tensor("bqk2", [D, 1], F32, kind="ExternalInput")
    out_ext = nc.dram_tensor("out", [TLOC, D], F32, kind="ExternalOutput")

    with tile.TileContext(nc) as tc:
        import contextlib

        with contextlib.ExitStack() as ctx:
            const = ctx.enter_context(tc.tile_pool(name="const", bufs=1))
            persist = ctx.enter_context(tc.tile_pool(name="persist", bufs=1))

            identity = const.tile([128, 128], BF16)
            make_identity(nc, identity[:])
            mbias = const.tile([128, 1], F32)
            nc.vector.memset(mbias[:], -2.0)
            bqk_sb = const.tile([128, DC, 1], F32)

            # persistent SBUF tensors
            # Q'^T fp8 {feat-pair x plane x q} for DR stage A
            qpt_sb = persist.tile([128, NPAIR, 2, TLOC], FP8)
            # att@[x|32] accumulator {qc x (e,den)}
            o_sb = persist.tile([128, TLOC // 128, 1026], BF16)
            rden_sb = persist.tile([128, TLOC // 128], F32)
            nc.vector.memset(o_sb[:], 0.0)
            # W_vo DMA is issued inside the super loop (after super-0 K/V)
            # so the startup queue serves phase 1's critical path first
            wvo_sb = persist.tile([128, NPAIR, 2, D], FP8)

            # attention pools opened before phase 1 so super-0 K/V DMAs
            # get disjoint SBUF addresses and prefetch during the Q' GEMM
            kvp = ctx.enter_context(tc.tile_pool(name="kv", bufs=2))
            ptp = ctx.enter_context(tc.tile_pool(name="pt", bufs=10))

            # ---------------- phase 1: Q'^T (local tokens) ----------------
            with nc.named_scope("p1_qproj"), \
                 tc.tile_pool(name="wqk", bufs=1) as wqkp, \
                 tc.tile_pool(name="xtl", bufs=2) as xtlp, \
                 tc.tile_pool(name="ps1", bufs=4, space="PSUM") as ps1:
                # DMA order = PE need order: xtl(ts0), then wqk pair by
                # pair (the first dc group consumes them in sequence),
                # then the tiny bias
                wqk_sb = wqkp.tile([128, NPAIR, 2, D], FP8)
                xts = []
                for ts in range(TLOC // TSUP):
                    xt_t = xtlp.tile([128, NPAIR, 2, TSUP], FP8,
                                     tag="xtl", name=f"xt{ts}")
                    xts.append(xt_t)
                nc.sync.dma_start(
                    xts[0][:],
                    xtl_f8[:, 0:TSUP].rearrange(
                        "(c q p) t -> p c q t", p=128, q=2))
                for pr in range(NPAIR):
                    nc.sync.dma_start(
                        wqk_sb[:, pr, :, :],
                        w_qk8[pr * 256:(pr + 1) * 256, :].rearrange(
                            "(q p) d -> p q d", p=128))
                nc.sync.dma_start(
                    bqk_sb[:], bqk2.ap().rearrange("(c p) o -> p c o",
                                                   p=128))
                for ts in range(TLOC // TSUP):
                    xt = xts[ts]
                    if ts > 0:
                        nc.sync.dma_start(
                            xt[:],
                            xtl_f8[:, ts * TSUP:(ts + 1) * TSUP].rearrange(
                                "(c q p) t -> p c q t", p=128, q=2))
                    for dc in range(DC):
                        qp = ps1.tile([128, TSUP], F32, tag="qp")
                        for pr in range(NPAIR):
                            nc.tensor.matmul(
                                qp[:],
                                lhsT=wqk_sb[:, pr, :,
                                            dc * 128:dc * 128 + 128],
                                rhs=xt[:, pr, :, :],
                                start=(pr == 0), stop=(pr == NPAIR - 1),
                                perf_mode=DR)
                        nc.vector.tensor_scalar_add(
                            qpt_sb[:, dc // 2, dc % 2,
                                   ts * TSUP:(ts + 1) * TSUP],
                            qp[:], bqk_sb[:, dc, :])

            # ---------------- phase 2: flash attention --------------------
            # stage-B output chunks: each must fit one PSUM bank (<=512
            # fp32); the last chunk carries the denominator column
            OCH = [(0, 512), (512, 768), (768, 1025)]
            with nc.named_scope("p2_attn"), \
                 tc.tile_pool(name="pso0", bufs=2, space="PSUM") as pso0, \
                 tc.tile_pool(name="pso1", bufs=2, space="PSUM") as pso1, \
                 tc.tile_pool(name="pso2", bufs=2, space="PSUM") as pso2, \
                 tc.tile_pool(name="psst", bufs=2, space="PSUM") as psst:
                opools = [pso0, pso1, pso2]
                for s in range(NSUP):
                    k_sb = kvp.tile([128, NPAIR, 2, KSUP], FP8, tag="k")
                    nc.sync.dma_start(
                        k_sb[:],
                        xt_f8[:, s * KSUP:(s + 1) * KSUP].rearrange(
                            "(c q p) t -> p c q t", p=128, q=2))
                    v_sb = kvp.tile([128, NG, 2, VW], FP8, tag="v")
                    nc.sync.dma_start(
                        v_sb[:],
                        xpad_f8[s * KSUP:(s + 1) * KSUP, :].rearrange(
                            "(g ko p) d -> p g ko d", p=128, ko=2))
                    if s == 0:
                        nc.sync.dma_start(
                            wvo_sb[:],
                            w_vo8.ap().rearrange("(c q p) d -> p c q d",
                                                 p=128, q=2))
                    for qb in range(TLOC // QBLK):
                        # stage A: S^T chunks (fp8 DR) -> exp(z/(32 s) - 2)
                        # -> fp8 P^T planes [Ki, Ko] (shift cancels in
                        # softmax; keeps exp under e4m3 max 448)
                        pts = []
                        for kc in range(KC):
                            if kc % 2 == 0:
                                pt_t = ptp.tile([128, 2, QBLK], FP8,
                                                tag="pt")
                                pts.append(pt_t)
                            st = psst.tile([128, QBLK], F32, tag="st")
                            for pr in range(NPAIR):
                                nc.tensor.matmul(
                                    st[:],
                                    lhsT=k_sb[:, pr, :,
                                              kc * 128:kc * 128 + 128],
                                    rhs=qpt_sb[:, pr, :,
                                               qb * QBLK:(qb + 1) * QBLK],
                                    start=(pr == 0), stop=(pr == NPAIR - 1),
                                    perf_mode=DR)
                            nc.scalar.activation(
                                pts[kc // 2][:, kc % 2, :], st[:], ActF.Exp,
                                bias=mbias[:, 0:1], scale=1.0 / (WSC * SCALE))
                        # stage B: att @ [x | 32]; column 1024 of the PSUM
                        # is 32*sum(P) = the softmax denominator
                        for sub in range(QBLK // 128):
                            qc = qb * (QBLK // 128) + sub
                            for ci, (w0, w1) in enumerate(OCH):
                                o_ps = opools[ci].tile(
                                    [128, w1 - w0], F32,
                                    tag="ops%d" % ci)
                                for g in range(NG):
                                    nc.tensor.matmul(
                                        o_ps[:],
                                        lhsT=pts[g][:, :, sub * 128:
                                                    (sub + 1) * 128],
                                        rhs=v_sb[:, g, :, w0:w1],
                                        start=(g == 0), stop=(g == NG - 1),
                                        perf_mode=DR)
                                nc.vector.tensor_add(
                                    o_sb[:, qc, w0:w1], o_ps[:],
                                    o_sb[:, qc, w0:w1])

            # ---------------- phase 3: out-proj + normalize + residual ----
            with nc.named_scope("p3_out"), \
                 tc.tile_pool(name="ot", bufs=1) as otp, \
                 tc.tile_pool(name="xr", bufs=2) as xrp, \
                 tc.tile_pool(name="fo", bufs=4) as fop, \
                 tc.tile_pool(name="pst", bufs=4, space="PSUM") as pstp, \
                 tc.tile_pool(name="psf", bufs=4, space="PSUM") as psfp:
                QC = TLOC // 128  # 8
                ot_sb = otp.tile([128, NPAIR, 2, TLOC], FP8)  # (att@x)^T

                for qc in range(QC):
                    nc.vector.reciprocal(rden_sb[:, qc:qc + 1],
                                         o_sb[:, qc, 1024:1025])
                    # 4 transposes batched per PSUM tile, drained by ONE
                    # Act-engine copy (Pool can't read PSUM; DVE stays
                    # free for the STTs); the x1/8 scale keeps the fp8
                    # copy under the e4m3 max of 448 (raw O reaches
                    # ~450); the den column of 4.0 (= 32/8) compensates
                    for dh in range(2):
                        tp = pstp.tile([128, 512], BF16, tag="tp")
                        for k in range(4):
                            dc = dh * 4 + k
                            nc.tensor.transpose(
                                tp[:, k * 128:(k + 1) * 128],
                                o_sb[:, qc, dc * 128:dc * 128 + 128],
                                identity[:])
                        nc.scalar.activation(
                            ot_sb[:, dh * 2:dh * 2 + 2, :,
                                  qc * 128:(qc + 1) * 128],
                            tp[:], ActF.Copy, scale=0.125)
                    xr = xrp.tile([128, D], F32, tag="xr")
                    nc.sync.dma_start(
                        xr[:], x_loc[qc * 128:(qc + 1) * 128, :])
                    for half in range(2):
                        fp = psfp.tile([128, 512], F32, tag="fp")
                        for pr in range(NPAIR):
                            nc.tensor.matmul(
                                fp[:],
                                lhsT=ot_sb[:, pr, :,
                                           qc * 128:(qc + 1) * 128],
                                rhs=wvo_sb[:, pr, :,
                                           half * 512:half * 512 + 512],
                                start=(pr == 0), stop=(pr == NPAIR - 1),
                                perf_mode=DR)
                        fo = fop.tile([128, 512], F32, tag="fo")
                        # psum = (O/8) @ (32 W_vo)^T = 4 O @ W_vo^T and
                        # rden = 1/(4 den), so out = psum*rden + x, fused
                        nc.vector.scalar_tensor_tensor(
                            fo[:], fp[:], rden_sb[:, qc:qc + 1],
                            xr[:, half * 512:half * 512 + 512],
                            op0=AluOp.mult, op1=AluOp.add)
                        nc.sync.dma_start(
                            out_ext[qc * 128:(qc + 1) * 128,
                                    half * 512:half * 512 + 512], fo[:])

    nc.compile()
    return nc


def _get_program():
    if "nc" not in _PROGRAM_CACHE:
        _PROGRAM_CACHE["nc"] = build_program()
    return _PROGRAM_CACHE["nc"]


def make_in_maps(x, Wq, bq, Wk, bk, Wv, bv, Wo, bo):
    """Host-side sharding/layout prep and weight folding (constant folding
    of D x D weight products -- all N-sized tensor math runs on device).
    Returns per-core input maps."""
    import ml_dtypes

    f8 = ml_dtypes.float8_e4m3fn
    x = np.ascontiguousarray(x, dtype=np.float32)
    x_f8 = x.astype(f8)
    xt_f8 = np.ascontiguousarray(x_f8.T)
    xpad_f8 = np.zeros((N, VW), dtype=f8)
    xpad_f8[:, :D] = x_f8
    xpad_f8[:, D] = f8(WSC / 8.0)  # den column; matches the /8 O scaling
    Wq64 = np.asarray(Wq, np.float64)
    Wk64 = np.asarray(Wk, np.float64)
    Wv64 = np.asarray(Wv, np.float64)
    Wo64 = np.asarray(Wo, np.float64)
    # z = q k^T = (x Wq^T + bq) Wk x^T + (q.bk) 1^T; the bk term is a
    # per-row constant -- softmax cancels it exactly, so K is dropped.
    w_qk8 = (WSC * (Wq64.T @ Wk64)).astype(np.float32).astype(f8)
    bqk = (WSC * (np.asarray(bq, np.float64) @ Wk64)).astype(np.float32)
    # att(x Wv^T + bv) Wo^T + bo = (att x)(Wo Wv)^T + (bo + Wo bv),
    # exact because att rows sum to 1 in the on-device normalization.
    w_vo8 = np.ascontiguousarray(
        (WSC * (Wo64 @ Wv64)).T.astype(np.float32)).astype(f8)
    boeff = (np.asarray(bo, np.float64)
             + Wo64 @ np.asarray(bv, np.float64)).astype(np.float32)
    in_maps = []
    for c in range(NCORES):
        sl = slice(c * TLOC, (c + 1) * TLOC)
        in_maps.append({
            "xt_f8": xt_f8,
            "xpad_f8": xpad_f8,
            "xtl_f8": np.ascontiguousarray(xt_f8[:, sl]),
            "x_loc": np.ascontiguousarray(x[sl, :] + boeff[None, :]),
            "w_qk8": w_qk8,
            "w_vo8": w_vo8,
            "bqk2": bqk.reshape(D, 1),
        })
    return in_maps


def kernel(x, Wq, bq, Wk, bk, Wv, bv, Wo, bo, _trace=False):
    from concourse.bass_utils import run_bass_kernel_spmd

    nc = _get_program()
    in_maps = make_in_maps(x, Wq, bq, Wk, bk, Wv, bv, Wo, bo)
    res = run_bass_kernel_spmd(nc, in_maps, list(range(NCORES)),
                               trace=_trace)
    out = np.concatenate([res.results[c]["out"] for c in range(NCORES)],
                         axis=0)
    if _trace:
        kernel.last_results = res
    return out


# revision 19
# speedup vs baseline: 1.6479x; 1.0415x over previous
"""Trainium2 Bass kernel for single-head dense attention.

Reference computation (all fp32):
    q = x @ Wq.T + bq ; k = x @ Wk.T + bk ; v = x @ Wv.T + bv      # [N, D]
    att = softmax((q @ k.T) / sqrt(128), axis=-1)                  # [N, N]
    out = (att @ v) @ Wo.T + bo + x                                # [N, D]

N = 8192, D = 1024, 8 NeuronCores.  Queries are sharded 8 ways; no
collectives needed.

Algebraic restructure (exact up to fp reassociation):
  * z = q @ k.T = (x Wq^T + bq) Wk x^T + (q . bk) 1^T.  The bk term adds a
    per-row constant, which softmax cancels exactly, so K IS NEVER
    COMPUTED.  Host folds W_qk = 32 Wq^T Wk and b_qk = 32 bq @ Wk (the x32
    keeps fp8 weights out of the denormal range; it cancels in softmax via
    the exp input scale).
  * att @ (x Wv^T + bv) Wo^T + bo = (att @ x) @ (Wo Wv)^T + (bo + Wo bv):
    the PV matmul consumes x directly (V never computed); host folds
    W_vo = 32 Wo @ Wv (x32 rescaled away in the output normalization) and
    bo_eff = bo + Wo @ bv (exact: att rows sum to 1).

All big GEMMs run in fp8e4m3 with DoubleRow perf mode (256-deep
contraction per instruction, 2x bf16 MAC throughput on HW).  Softmax
denominators come for free: the fp8 V matrix gets an extra column of
32.0, so column 1024 of the PV output is 32*sum_k(P) and the final
normalization uses 1/(32 den) directly.

Per-core program (Tile framework):
  phase 1: Q'^T [D, 1024] fp8 (DR GEMM on local tokens, psum f32 + bias,
           written as fp8 in (pair, plane) layout for stage A)
  phase 2: flash attention over key supers of 1024 in S^T layout (keys on
           partitions).  Per (super, 512-query block): stage A computes
           S^T chunks [128k, 512q] via fp8 DR and exps them (scale folded,
           -2 shift keeps exp under e4m3 max 448) into fp8 P^T tiles;
           stage B runs (att @ [x | 32]) via fp8 DR with P^T chunks as
           stationary operands; the PSUM is accumulated into bf16 o_sb
           (denominator column included) by the DVE.
  phase 3: PE-transpose O, @ W_vo^T (fp8 DR), then one fused DVE op
           per tile: out = psum * (1/(32 den)) + (x + bo_eff).
"""

import sys

if "/opt/trn_rl_repo" not in sys.path:
    sys.path.insert(0, "/opt/trn_rl_repo")

import numpy as np

import concourse.bass as bass
import concourse.tile as tile
from concourse import bacc, mybir
from concourse.masks import make_identity

N = 8192
D = 1024
NCORES = 8
TLOC = N // NCORES  # 1024 tokens per core
SCALE = float(np.sqrt(128.0))
WSC = 32.0            # fp8 weight pre-scale (denormal avoidance)
F32 = mybir.dt.float32
BF16 = mybir.dt.bfloat16
FP8 = mybir.dt.float8e4
DR = mybir.MatmulPerfMode.DoubleRow
ActF = mybir.ActivationFunctionType
AluOp = mybir.AluOpType

KSUP = 1024           # keys per attention super-block
NSUP = N // KSUP      # 8
TSUP = 512            # token block in phase 1
QBLK = 512            # query columns per S^T matmul
DC = D // 128         # 8 feature chunks
NPAIR = DC // 2       # 4 DoubleRow feature-pair chunks
VW = 1032             # padded V width: 1024 features + 32-col + pad
KC = KSUP // 128      # 8 key chunks per super
NG = KSUP // 256      # 4 DoubleRow key groups per super

_PROGRAM_CACHE = {}


def build_program():
    nc = bacc.Bacc("TRN2", target_bir_lowering=False, debug=False,
                   num_devices=NCORES)

    xt_f8 = nc.dram_tensor("xt_f8", [D, N], FP8, kind="ExternalInput")
    xpad_f8 = nc.dram_tensor("xpad_f8", [N, VW], FP8, kind="ExternalInput")
    xtl_f8 = nc.dram_tensor("xtl_f8", [D, TLOC], FP8, kind="ExternalInput")
    x_loc = nc.dram_tensor("x_loc", [TLOC, D], F32, kind="ExternalInput")
    w_qk8 = nc.dram_tensor("w_qk8", [D, D], FP8, kind="ExternalInput")
    w_vo8 = nc.dram_tensor("w_vo8", [D, D], FP8, kind="ExternalInput")
    bqk2 = nc.dram_tensor("bqk2", [D, 1], F32, kind="ExternalInput")
    out_ext = nc.dram_tensor("out", [TLOC, D], F32, kind="ExternalOutput")

    with tile.TileContext(nc) as tc:
        import contextlib

        with contextlib.ExitStack() as ctx:
            const = ctx.enter_context(tc.tile_pool(name="const", bufs=1))
            persist = ctx.enter_context(tc.tile_pool(name="persist", bufs=1))

            identity = const.tile([128, 128], BF16)
            make_identity(nc, identity[:])
            mbias = const.tile([128, 1], F32)
            nc.vector.memset(mbias[:], -2.0)
            bqk_sb = const.tile([128, DC, 1], F32)

            # persistent SBUF tensors
            # Q'^T fp8 {feat-pair x plane x q} for DR stage A
            qpt_sb = persist.tile([128, NPAIR, 2, TLOC], FP8)
            # att@[x|32] accumulator {qc x (e,den)}
            o_sb = persist.tile([128, TLOC // 128, 1026], BF16)
            rden_sb = persist.tile([128, TLOC // 128], F32)
            nc.vector.memset(o_sb[:], 0.0)
            # W_vo / residual DMAs are issued inside the super loop (after
            # super-0 K/V) so the startup queue serves phase 1's critical
            # path first; both trickle in during phase 2
            wvo_sb = persist.tile([128, NPAIR, 2, D], FP8)
            xres_sb = persist.tile([128, TLOC // 128, D], F32)

            # attention pools opened before phase 1 so super-0 K/V DMAs
            # get disjoint SBUF addresses and prefetch during the Q' GEMM
            kvp = ctx.enter_context(tc.tile_pool(name="kv", bufs=2))
            ptp = ctx.enter_context(tc.tile_pool(name="pt", bufs=10))

            # ---------------- phase 1: Q'^T (local tokens) ----------------
            with nc.named_scope("p1_qproj"), \
                 tc.tile_pool(name="wqk", bufs=1) as wqkp, \
                 tc.tile_pool(name="xtl", bufs=2) as xtlp, \
                 tc.tile_pool(name="ps1", bufs=4, space="PSUM") as ps1:
                # DMA order = PE need order: xtl(ts0), then wqk pair by
                # pair (the first dc group consumes them in sequence),
                # then the tiny bias
                wqk_sb = wqkp.tile([128, NPAIR, 2, D], FP8)
                xts = []
                for ts in range(TLOC // TSUP):
                    xt_t = xtlp.tile([128, NPAIR, 2, TSUP], FP8,
                                     tag="xtl", name=f"xt{ts}")
                    xts.append(xt_t)
                nc.sync.dma_start(
                    xts[0][:],
                    xtl_f8[:, 0:TSUP].rearrange(
                        "(c q p) t -> p c q t", p=128, q=2))
                for pr in range(NPAIR):
                    nc.sync.dma_start(
                        wqk_sb[:, pr, :, :],
                        w_qk8[pr * 256:(pr + 1) * 256, :].rearrange(
                            "(q p) d -> p q d", p=128))
                nc.sync.dma_start(
                    bqk_sb[:], bqk2.ap().rearrange("(c p) o -> p c o",
                                                   p=128))
                for ts in range(TLOC // TSUP):
                    xt = xts[ts]
                    if ts > 0:
                        nc.sync.dma_start(
                            xt[:],
                            xtl_f8[:, ts * TSUP:(ts + 1) * TSUP].rearrange(
                                "(c q p) t -> p c q t", p=128, q=2))
                    for dc in range(DC):
                        qp = ps1.tile([128, TSUP], F32, tag="qp")
                        for pr in range(NPAIR):
                            nc.tensor.matmul(
                                qp[:],
                                lhsT=wqk_sb[:, pr, :,
                                            dc * 128:dc * 128 + 128],
                                rhs=xt[:, pr, :, :],
                                start=(pr == 0), stop=(pr == NPAIR - 1),
                                perf_mode=DR)
                        nc.vector.tensor_scalar_add(
                            qpt_sb[:, dc // 2, dc % 2,
                                   ts * TSUP:(ts + 1) * TSUP],
                            qp[:], bqk_sb[:, dc, :])

            # ---------------- phase 2: flash attention --------------------
            # stage-B output chunks: each must fit one PSUM bank (<=512
            # fp32); the last chunk carries the denominator column
            OCH = [(0, 512), (512, 768), (768, 1025)]
            with nc.named_scope("p2_attn"), \
                 tc.tile_pool(name="pso0", bufs=2, space="PSUM") as pso0, \
                 tc.tile_pool(name="pso1", bufs=2, space="PSUM") as pso1, \
                 tc.tile_pool(name="pso2", bufs=2, space="PSUM") as pso2, \
                 tc.tile_pool(name="psst", bufs=2, space="PSUM") as psst:
                opools = [pso0, pso1, pso2]
                for s in range(NSUP):
                    k_sb = kvp.tile([128, NPAIR, 2, KSUP], FP8, tag="k")
                    nc.sync.dma_start(
                        k_sb[:],
                        xt_f8[:, s * KSUP:(s + 1) * KSUP].rearrange(
                            "(c q p) t -> p c q t", p=128, q=2))
                    v_sb = kvp.tile([128, NG, 2, VW], FP8, tag="v")
                    nc.sync.dma_start(
                        v_sb[:],
                        xpad_f8[s * KSUP:(s + 1) * KSUP, :].rearrange(
                            "(g ko p) d -> p g ko d", p=128, ko=2))
                    if s == 0:
                        nc.sync.dma_start(
                            wvo_sb[:],
                            w_vo8.ap().rearrange("(c q p) d -> p c q d",
                                                 p=128, q=2))
                        nc.sync.dma_start(
                            xres_sb[:],
                            x_loc.ap().rearrange("(c p) d -> p c d",
                                                 p=128))
                    for qb in range(TLOC // QBLK):
                        # stage A: S^T chunks (fp8 DR) -> exp(z/(32 s) - 2)
                        # -> fp8 P^T planes [Ki, Ko] (shift cancels in
                        # softmax; keeps exp under e4m3 max 448)
                        pts = []
                        for kc in range(KC):
                            if kc % 2 == 0:
                                pt_t = ptp.tile([128, 2, QBLK], FP8,
                                                tag="pt")
                                pts.append(pt_t)
                            st = psst.tile([128, QBLK], F32, tag="st")
                            for pr in range(NPAIR):
                                nc.tensor.matmul(
                                    st[:],
                                    lhsT=k_sb[:, pr, :,
                                              kc * 128:kc * 128 + 128],
                                    rhs=qpt_sb[:, pr, :,
                                               qb * QBLK:(qb + 1) * QBLK],
                                    start=(pr == 0), stop=(pr == NPAIR - 1),
                                    perf_mode=DR)
                            nc.scalar.activation(
                                pts[kc // 2][:, kc % 2, :], st[:], ActF.Exp,
                                bias=mbias[:, 0:1], scale=1.0 / (WSC * SCALE))
                        # stage B: att @ [x | 32]; column 1024 of the PSUM
                        # is 32*sum(P) = the softmax denominator
                        for sub in range(QBLK // 128):
                            qc = qb * (QBLK // 128) + sub
                            for ci, (w0, w1) in enumerate(OCH):
                                o_ps = opools[ci].tile(
                                    [128, w1 - w0], F32,
                                    tag="ops%d" % ci)
                                for g in range(NG):
                                    nc.tensor.matmul(
                                        o_ps[:],
                                        lhsT=pts[g][:, :, sub * 128:
                                                    (sub + 1) * 128],
                                        rhs=v_sb[:, g, :, w0:w1],
                                        start=(g == 0), stop=(g == NG - 1),
                                        perf_mode=DR)
                                nc.vector.tensor_add(
                                    o_sb[:, qc, w0:w1], o_ps[:],
                                    o_sb[:, qc, w0:w1])

            # ---------------- phase 3: out-proj + normalize + residual ----
            with nc.named_scope("p3_out"), \
                 tc.tile_pool(name="ot", bufs=1) as otp, \
                 tc.tile_pool(name="fo", bufs=4) as fop, \
                 tc.tile_pool(name="pst", bufs=4, space="PSUM") as pstp, \
                 tc.tile_pool(name="psf", bufs=4, space="PSUM") as psfp:
                QC = TLOC // 128  # 8
                ot_sb = otp.tile([128, NPAIR, 2, TLOC], FP8)  # (att@x)^T

                # pass 1: transposes back-to-back on the PE, 4 batched
                # per PSUM tile, each drained by ONE Act-engine copy (so
                # the pass-2 GEMMs never wait behind queued transposes);
                # the x1/8 scale keeps the fp8 copy under the e4m3 max
                # of 448 (raw O reaches ~450); the den column of 4.0
                # (= 32/8) compensates exactly
                for qc in range(QC):
                    nc.vector.reciprocal(rden_sb[:, qc:qc + 1],
                                         o_sb[:, qc, 1024:1025])
                    for dh in range(2):
                        tp = pstp.tile([128, 512], BF16, tag="tp")
                        for k in range(4):
                            dc = dh * 4 + k
                            nc.tensor.transpose(
                                tp[:, k * 128:(k + 1) * 128],
                                o_sb[:, qc, dc * 128:dc * 128 + 128],
                                identity[:])
                        nc.scalar.activation(
                            ot_sb[:, dh * 2:dh * 2 + 2, :,
                                  qc * 128:(qc + 1) * 128],
                            tp[:], ActF.Copy, scale=0.125)
                # pass 2: output projection + normalize + residual
                for qc in range(QC):
                    for half in range(2):
                        fp = psfp.tile([128, 512], F32, tag="fp")
                        for pr in range(NPAIR):
                            nc.tensor.matmul(
                                fp[:],
                                lhsT=ot_sb[:, pr, :,
                                           qc * 128:(qc + 1) * 128],
                                rhs=wvo_sb[:, pr, :,
                                           half * 512:half * 512 + 512],
                                start=(pr == 0), stop=(pr == NPAIR - 1),
                                perf_mode=DR)
                        fo = fop.tile([128, 512], F32, tag="fo")
                        # psum = (O/8) @ (32 W_vo)^T = 4 O @ W_vo^T and
                        # rden = 1/(4 den), so out = psum*rden + x, fused
                        nc.vector.scalar_tensor_tensor(
                            fo[:], fp[:], rden_sb[:, qc:qc + 1],
                            xres_sb[:, qc, half * 512:half * 512 + 512],
                            op0=AluOp.mult, op1=AluOp.add)
                        nc.sync.dma_start(
                            out_ext[qc * 128:(qc + 1) * 128,
                                    half * 512:half * 512 + 512], fo[:])

    nc.compile()
    return nc


def _get_program():
    if "nc" not in _PROGRAM_CACHE:
        _PROGRAM_CACHE["nc"] = build_program()
    return _PROGRAM_CACHE["nc"]


def make_in_maps(x, Wq, bq, Wk, bk, Wv, bv, Wo, bo):
    """Host-side sharding/layout prep and weight folding (constant folding
    of D x D weight products -- all N-sized tensor math runs on device).
    Returns per-core input maps."""
    import ml_dtypes

    f8 = ml_dtypes.float8_e4m3fn
    x = np.ascontiguousarray(x, dtype=np.float32)
    x_f8 = x.astype(f8)
    xt_f8 = np.ascontiguousarray(x_f8.T)
    xpad_f8 = np.zeros((N, VW), dtype=f8)
    xpad_f8[:, :D] = x_f8
    xpad_f8[:, D] = f8(WSC / 8.0)  # den column; matches the /8 O scaling
    Wq64 = np.asarray(Wq, np.float64)
    Wk64 = np.asarray(Wk, np.float64)
    Wv64 = np.asarray(Wv, np.float64)
    Wo64 = np.asarray(Wo, np.float64)
    # z = q k^T = (x Wq^T + bq) Wk x^T + (q.bk) 1^T; the bk term is a
    # per-row constant -- softmax cancels it exactly, so K is dropped.
    w_qk8 = (WSC * (Wq64.T @ Wk64)).astype(np.float32).astype(f8)
    bqk = (WSC * (np.asarray(bq, np.float64) @ Wk64)).astype(np.float32)
    # att(x Wv^T + bv) Wo^T + bo = (att x)(Wo Wv)^T + (bo + Wo bv),
    # exact because att rows sum to 1 in the on-device normalization.
    w_vo8 = np.ascontiguousarray(
        (WSC * (Wo64 @ Wv64)).T.astype(np.float32)).astype(f8)
    boeff = (np.asarray(bo, np.float64)
             + Wo64 @ np.asarray(bv, np.float64)).astype(np.float32)
    in_maps = []
    for c in range(NCORES):
        sl = slice(c * TLOC, (c + 1) * TLOC)
        in_maps.append({
            "xt_f8": xt_f8,
            "xpad_f8": xpad_f8,
            "xtl_f8": np.ascontiguousarray(xt_f8[:, sl]),
            "x_loc": np.ascontiguousarray(x[sl, :] + boeff[None, :]),
            "w_qk8": w_qk8,
            "w_vo8": w_vo8,
            "bqk2": bqk.reshape(D, 1),
        })
    return in_maps


def kernel(x, Wq, bq, Wk, bk, Wv, bv, Wo, bo, _trace=False):
    from concourse.bass_utils import run_bass_kernel_spmd

    nc = _get_program()
    in_maps = make_in_maps(x, Wq, bq, Wk, bk, Wv, bv, Wo, bo)
    res = run_bass_kernel_spmd(nc, in_maps, list(range(NCORES)),
                               trace=_trace)
    out = np.concatenate([res.results[c]["out"] for c in range(NCORES)],
                         axis=0)
    if _trace:
        kernel.last_results = res
    return out
